# revision 5
# baseline (speedup 1.0000x reference)
"""Trainium2 Bass kernel for nn_KnowledgeIntegrationLoss.

Computes (reference semantics):
    x = [new_knowledge; existing_knowledge]            # [11, 8192]
    E = MLP_encoder(x)                                 # [11, 128] (3 Linear, ReLU x2)
    geo_j = ||E_0 - E_j||, cos_j = <E_0,E_j>/(max(|E_0|,eps)*max(|E_j|,eps))
    avg = mean_{j=1..10}(geo_j - cos_j)
    q = MLP_integrator([E_0; math_metrics])            # [1]
    out = avg + exp(-q)                                # scalar

Distribution (8 NeuronCores, no device collectives — launch skew across the
axon-tunneled cores makes any cross-core dependency cost 20-250us):
  Phase 1 (SPMD x8): column-shard W1: core j computes
      h1T_j = ReLU(x @ W1[:, 64j:64j+64] + b1[64j:64j+64]).T   -> [64, 11]
  Host: pure concatenation/layout shuffle of the 8 blocks (zero FLOPs).
  Phase 2 (1 core): layers 2..3 + loss tail on device -> scalar.
"""

import numpy as np

import concourse.bass as bass
import concourse.mybir as mybir
import concourse.tile as tile
from concourse import bacc
from concourse import bass_utils

F32 = mybir.dt.float32
N_CORES = 8
KDIM = 8192
EPS = 1e-8

# ---------------------------------------------------------------------------
# host-side layout helpers (pure reshapes/transposes, no FLOPs)
# ---------------------------------------------------------------------------


def _kmajor_image(a, p=128):
    """[K, M] (K = c*p + part) -> SBUF image [p, (K//p)*M]: img[part, c*M+m] = a[c*p+part, m]."""
    K, M = a.shape
    n = K // p
    return np.ascontiguousarray(a.reshape(n, p, M).transpose(1, 0, 2).reshape(p, n * M))


# ---------------------------------------------------------------------------
# phase 1: per-core first-layer column shard
# ---------------------------------------------------------------------------

N_W1_CHUNKS = 8  # DMA W1 shard in chunks so matmuls chase the DMA


def build_phase1():
    nc = bacc.Bacc("TRN2", target_bir_lowering=False, debug=False,
                   num_devices=N_CORES)

    x_img = nc.dram_tensor("x_img", [128, 64 * 11], F32, kind="ExternalInput")
    w1_img = nc.dram_tensor("w1_img", [128, 64 * 64], F32, kind="ExternalInput")
    b1_col = nc.dram_tensor("b1_col", [64, 1], F32, kind="ExternalInput")
    ident = nc.dram_tensor("ident", [11, 11], F32, kind="ExternalInput")
    h1t_out = nc.dram_tensor("h1t_out", [64, 11], F32, kind="ExternalOutput")

    with tile.TileContext(nc) as tc:
        with (
            tc.tile_pool(name="sbuf", bufs=1) as sb,
            tc.tile_pool(name="psum", bufs=1, space="PSUM") as ps,
        ):
            xsb = sb.tile([128, 64 * 11], F32, tag="xsb")
            nc.sync.dma_start(xsb[:], x_img[:])
            idsb = sb.tile([11, 11], F32, tag="idsb")
            nc.sync.dma_start(idsb[:], ident[:])
            b1sb = sb.tile([64, 1], F32, tag="b1sb")
            nc.sync.dma_start(b1sb[:], b1_col[:])

            w1sb = []
            cw = (64 * 64) // N_W1_CHUNKS  # columns per chunk
            for c in range(N_W1_CHUNKS):
                t = sb.tile([128, cw], F32, tag=f"w1_{c}")
                nc.sync.dma_start(t[:], w1_img[:, c * cw:(c + 1) * cw])
                w1sb.append(t)

            psum1 = ps.tile([11, 64], F32, tag="psum1")
            per_chunk = 64 // N_W1_CHUNKS  # k-tiles per chunk
            for n in range(64):
                c, i = divmod(n, per_chunk)
                nc.tensor.matmul(
                    psum1[:, :],
                    xsb[:, 11 * n:11 * (n + 1)],
                    w1sb[c][:, 64 * i:64 * (i + 1)],
                    start=(n == 0),
                    stop=(n == 63),
                )

            pre = sb.tile([11, 64], F32, tag="pre")
            nc.vector.tensor_copy(pre[:], psum1[:])
            psT = ps.tile([64, 11], F32, tag="psT")
            nc.tensor.transpose(psT[:], pre[:], idsb[:])
            h1t = sb.tile([64, 11], F32, tag="h1t")
            nc.scalar.activation(h1t[:], psT[:],
                                 mybir.ActivationFunctionType.Relu,
                                 bias=b1sb[:, 0:1])
            nc.sync.dma_start(h1t_out[:], h1t[:])
    nc.compile()
    return nc


def phase1_inputs(x, W1, b1):
    """Per-core input maps for phase 1. x: [11, 8192]."""
    xT = _kmajor_image(np.ascontiguousarray(x.T))         # [128, 704]
    ident = np.eye(11, dtype=np.float32)
    maps = []
    for j in range(N_CORES):
        w1j = np.ascontiguousarray(W1[:, 64 * j:64 * (j + 1)])  # [8192, 64]
        maps.append({
            "x_img": xT,
            "w1_img": _kmajor_image(w1j),                 # [128, 4096]
            "b1_col": np.ascontiguousarray(
                b1[64 * j:64 * (j + 1)].reshape(64, 1)),
            "ident": ident,
        })
    return maps


# ---------------------------------------------------------------------------
# phase 2: layers 2..3 + loss tail, single core
# ---------------------------------------------------------------------------


def build_phase2():
    nc = bacc.Bacc("TRN2", target_bir_lowering=False, debug=False, num_devices=1)

    h1t_img = nc.dram_tensor("h1t_img", [128, 4 * 11], F32, kind="ExternalInput")
    w2_img = nc.dram_tensor("w2_img", [128, 4 * 256], F32, kind="ExternalInput")
    consts = nc.dram_tensor("consts", [128, CONST_COLS], F32, kind="ExternalInput")
    out = nc.dram_tensor("out", [1, 1], F32, kind="ExternalOutput")

    AF = mybir.ActivationFunctionType

    with tile.TileContext(nc) as tc:
        with (
            tc.tile_pool(name="sbuf", bufs=1) as sb,
            tc.tile_pool(name="psum", bufs=2, space="PSUM") as ps,
        ):
            h1sb = sb.tile([128, 44], F32, tag="h1sb")
            nc.sync.dma_start(h1sb[:], h1t_img[:])
            w2sb = sb.tile([128, 1024], F32, tag="w2sb")
            nc.sync.dma_start(w2sb[:], w2_img[:])
            csb = sb.tile([128, CONST_COLS], F32, tag="csb")
            nc.sync.dma_start(csb[:], consts[:])

            def cslice(name, np_, nf):
                c0 = CONST_OFF[name]
                return csb[0:np_, c0:c0 + nf]

            w3sb = cslice("w3", 128, 256)
            b2sb = cslice("b2", 128, 2)
            b3sb = cslice("b3", 11, 128)
            idsb = cslice("ident", 11, 11)
            onsb = cslice("ones", 1, 11)
            wmsb = cslice("wmean", 11, 1)
            wi1asb = cslice("wi1a", 128, 64)
            wi1bsb = cslice("wi1b", 6, 64)
            mm6sb = cslice("mm6", 6, 1)
            bi1sb = cslice("bi1", 1, 64)
            wi2sb = cslice("wi2", 64, 32)
            bi2sb = cslice("bi2", 1, 32)
            wi3sb = cslice("wi3", 32, 1)
            bi3sb = cslice("bi3", 1, 1)

            # ---- layer 2: h2 = ReLU(h1 @ W2 + b2), via h2T [2x(128,11)]
            psum2 = ps.tile([11, 256], F32, tag="pA")
            for t in range(4):
                nc.tensor.matmul(
                    psum2[:, :],
                    h1sb[:, 11 * t:11 * (t + 1)],
                    w2sb[:, 256 * t:256 * (t + 1)],
                    start=(t == 0),
                    stop=(t == 3),
                )
            h2pre = sb.tile([11, 256], F32, tag="h2pre")
            nc.vector.tensor_copy(h2pre[:], psum2[:])
            h2t = sb.tile([128, 22], F32, tag="h2t")
            for t in range(2):
                pst = ps.tile([128, 11], F32, tag="pT")
                nc.tensor.transpose(pst[:], h2pre[:, 128 * t:128 * (t + 1)], idsb)
                nc.scalar.activation(h2t[:, 11 * t:11 * (t + 1)], pst[:],
                                     AF.Relu, bias=b2sb[:, t:t + 1])

            # ---- layer 3: E = h2 @ W3 + b3  -> [11, 128]
            psum3 = ps.tile([11, 128], F32, tag="pA")
            for t in range(2):
                nc.tensor.matmul(
                    psum3[:, :],
                    h2t[:, 11 * t:11 * (t + 1)],
                    w3sb[:, 128 * t:128 * (t + 1)],
                    start=(t == 0),
                    stop=(t == 1),
                )
            Esb = sb.tile([11, 128], F32, tag="Esb")
            nc.vector.tensor_add(Esb[:], psum3[:], b3sb)

            # ---- broadcast row 0 (new_enc) to all 11 partitions
            bcN = ps.tile([11, 128], F32, tag="pA")
            nc.tensor.matmul(bcN[:, :], onsb, Esb[0:1, :], start=True, stop=True)

            # ---- per-row reductions: n2 = sum E^2, d = sum E*bcN, g2 = sum (E-bcN)^2
            scratch = sb.tile([11, 128], F32, tag="scratch")
            n2 = sb.tile([11, 1], F32, tag="n2")
            nc.vector.tensor_mul(scratch[:], Esb[:], Esb[:])
            nc.vector.tensor_reduce(n2[:], scratch[:], mybir.AxisListType.X,
                                    mybir.AluOpType.add)
            scratch2 = sb.tile([11, 128], F32, tag="scratch2")
            dvec = sb.tile([11, 1], F32, tag="dvec")
            nc.vector.tensor_mul(scratch2[:], Esb[:], bcN[:])
            nc.vector.tensor_reduce(dvec[:], scratch2[:], mybir.AxisListType.X,
                                    mybir.AluOpType.add)
            diff = sb.tile([11, 128], F32, tag="diff")
            nc.vector.tensor_sub(diff[:], Esb[:], bcN[:])
            scratch3 = sb.tile([11, 128], F32, tag="scratch3")
            g2 = sb.tile([11, 1], F32, tag="g2")
            nc.vector.tensor_mul(scratch3[:], diff[:], diff[:])
            nc.vector.tensor_reduce(g2[:], scratch3[:], mybir.AxisListType.X,
                                    mybir.AluOpType.add)

            # ---- norms, clamps, cos, geo
            nrm = sb.tile([11, 1], F32, tag="nrm")
            nc.scalar.activation(nrm[:], n2[:], AF.Sqrt)
            nmax = sb.tile([11, 1], F32, tag="nmax")
            nc.vector.tensor_scalar_max(nmax[:], nrm[:], EPS)
            geo = sb.tile([11, 1], F32, tag="geo")
            nc.scalar.activation(geo[:], g2[:], AF.Sqrt)

            bc0 = ps.tile([11, 1], F32, tag="pB")
            nc.tensor.matmul(bc0[:, :], onsb, nmax[0:1, 0:1], start=True, stop=True)
            denom = sb.tile([11, 1], F32, tag="denom")
            nc.vector.tensor_mul(denom[:], bc0[:], nmax[:])
            rden = sb.tile([11, 1], F32, tag="rden")
            nc.vector.reciprocal(rden[:], denom[:])
            cosv = sb.tile([11, 1], F32, tag="cosv")
            nc.vector.tensor_mul(cosv[:], dvec[:], rden[:])
            score = sb.tile([11, 1], F32, tag="score")
            nc.vector.tensor_sub(score[:], geo[:], cosv[:])

            # mean over rows 1..10 = wmean . score (wmean = [0, 0.1 x10])
            meanp = ps.tile([1, 1], F32, tag="pC")
            nc.tensor.matmul(meanp[:, :], score[:, 0:1], wmsb, start=True, stop=True)
            meansb = sb.tile([1, 1], F32, tag="meansb")
            nc.vector.tensor_copy(meansb[:], meanp[:])

            # ---- integrator MLP on [E_0; math_metrics]
            newT = ps.tile([128, 1], F32, tag="pB")
            nc.tensor.transpose(newT[:], Esb[0:1, :], idsb[0:1, 0:1])
            newTsb = sb.tile([128, 1], F32, tag="newTsb")
            nc.vector.tensor_copy(newTsb[:], newT[:])

            i1p = ps.tile([1, 64], F32, tag="pC")
            nc.tensor.matmul(i1p[:, :], newTsb[:, 0:1], wi1asb,
                             start=True, stop=False)
            nc.tensor.matmul(i1p[:, :], mm6sb, wi1bsb, start=False, stop=True)
            i1b = sb.tile([1, 64], F32, tag="i1b")
            nc.vector.tensor_add(i1b[:], i1p[:], bi1sb)
            i1r = sb.tile([1, 64], F32, tag="i1r")
            nc.vector.tensor_relu(i1r[:], i1b[:])

            i1T = ps.tile([64, 1], F32, tag="pB")
            nc.tensor.transpose(i1T[:], i1r[:], idsb[0:1, 0:1])
            i1Tsb = sb.tile([64, 1], F32, tag="i1Tsb")
            nc.vector.tensor_copy(i1Tsb[:], i1T[:])

            i2p = ps.tile([1, 32], F32, tag="pC")
            nc.tensor.matmul(i2p[:, :], i1Tsb[:, 0:1], wi2sb, start=True, stop=True)
            i2b = sb.tile([1, 32], F32, tag="i2b")
            nc.vector.tensor_add(i2b[:], i2p[:], bi2sb)
            i2r = sb.tile([1, 32], F32, tag="i2r")
            nc.vector.tensor_relu(i2r[:], i2b[:])

            i2T = ps.tile([32, 1], F32, tag="pB")
            nc.tensor.transpose(i2T[:], i2r[:], idsb[0:1, 0:1])
            i2Tsb = sb.tile([32, 1], F32, tag="i2Tsb")
            nc.vector.tensor_copy(i2Tsb[:], i2T[:])

            qp = ps.tile([1, 1], F32, tag="pC")
            nc.tensor.matmul(qp[:, :], i2Tsb[:, 0:1], wi3sb, start=True, stop=True)
            qb = sb.tile([1, 1], F32, tag="qb")
            nc.vector.tensor_add(qb[:], qp[:], bi3sb)
            il = sb.tile([1, 1], F32, tag="il")
            nc.scalar.activation(il[:], qb[:], AF.Exp, scale=-1.0)

            total = sb.tile([1, 1], F32, tag="total")
            nc.vector.tensor_add(total[:], il[:], meansb[:])
            nc.sync.dma_start(out[:], total[:])
    nc.compile()
    return nc


# const image layout: name -> (partitions, cols)
_CONST_SHAPES = [
    ("w3", 128, 256), ("b2", 128, 2), ("b3", 11, 128), ("ident", 11, 11),
    ("ones", 1, 11), ("wmean", 11, 1), ("wi1a", 128, 64), ("wi1b", 6, 64),
    ("mm6", 6, 1), ("bi1", 1, 64), ("wi2", 64, 32), ("bi2", 1, 32),
    ("wi3", 32, 1), ("bi3", 1, 1),
]
CONST_OFF = {}
_c = 0
for _n, _p, _f in _CONST_SHAPES:
    CONST_OFF[_n] = _c
    _c += _f
CONST_COLS = _c


def phase2_inputs(h1t_full, W2, b2, W3, b3, Wi1, bi1, Wi2, bi2, Wi3, bi3,
                  math_metrics):
    """h1t_full: [512, 11] = concat of the 8 per-core [64, 11] phase-1 outputs."""
    wm = np.zeros((11, 1), np.float32)
    wm[1:, 0] = 0.1
    vals = {
        "w3": _kmajor_image(W3),
        "b2": b2.reshape(2, 128).T,
        "b3": np.tile(b3, (11, 1)),
        "ident": np.eye(11, dtype=np.float32),
        "ones": np.ones((1, 11), np.float32),
        "wmean": wm,
        "wi1a": Wi1[:128],
        "wi1b": Wi1[128:],
        "mm6": math_metrics.reshape(6, 1),
        "bi1": bi1.reshape(1, 64),
        "wi2": Wi2,
        "bi2": bi2.reshape(1, 32),
        "wi3": Wi3,
        "bi3": bi3.reshape(1, 1),
    }
    consts = np.zeros((128, CONST_COLS), np.float32)
    for name, p, f in _CONST_SHAPES:
        v = np.asarray(vals[name], np.float32)
        assert v.shape == (p, f), (name, v.shape, (p, f))
        consts[:p, CONST_OFF[name]:CONST_OFF[name] + f] = v
    return {
        "h1t_img": _kmajor_image(h1t_full),   # [128, 44]
        "w2_img": _kmajor_image(W2),          # [128, 1024]
        "consts": consts,
    }


# ---------------------------------------------------------------------------
# entry point
# ---------------------------------------------------------------------------

_NC1 = None
_NC2 = None


def _get_ncs():
    global _NC1, _NC2
    if _NC1 is None:
        _NC1 = build_phase1()
        _NC2 = build_phase2()
    return _NC1, _NC2


def kernel(new_knowledge, existing_knowledge, math_metrics,
           W1, b1, W2, b2, W3, b3, Wi1, bi1, Wi2, bi2, Wi3, bi3):
    args = [new_knowledge, existing_knowledge, math_metrics,
            W1, b1, W2, b2, W3, b3, Wi1, bi1, Wi2, bi2, Wi3, bi3]
    (new_knowledge, existing_knowledge, math_metrics,
     W1, b1, W2, b2, W3, b3, Wi1, bi1, Wi2, bi2, Wi3, bi3) = [
        np.asarray(a, np.float32) for a in args]

    nc1, nc2 = _get_ncs()

    x = np.concatenate([new_knowledge[None, :], existing_knowledge], axis=0)
    maps1 = phase1_inputs(x, W1, b1)
    res1 = bass_utils.run_bass_kernel_spmd(
        nc1, maps1, core_ids=list(range(N_CORES)))
    # pure gather: concat per-core transposed h1 blocks -> [512, 11]
    h1t_full = np.concatenate(
        [res1.results[j]["h1t_out"] for j in range(N_CORES)], axis=0)

    maps2 = [phase2_inputs(h1t_full, W2, b2, W3, b3,
                           Wi1, bi1, Wi2, bi2, Wi3, bi3, math_metrics)]
    res2 = bass_utils.run_bass_kernel_spmd(nc2, maps2, core_ids=[0])
    return res2.results[0]["out"].reshape(()).astype(np.float32)


# revision 6
# speedup vs baseline: 1.0522x; 1.0522x over previous
"""Trainium2 Bass kernel for nn_KnowledgeIntegrationLoss.

Computes (reference semantics):
    x = [new_knowledge; existing_knowledge]            # [11, 8192]
    E = MLP_encoder(x)                                 # [11, 128] (3 Linear, ReLU x2)
    geo_j = ||E_0 - E_j||, cos_j = <E_0,E_j>/(max(|E_0|,eps)*max(|E_j|,eps))
    avg = mean_{j=1..10}(geo_j - cos_j)
    q = MLP_integrator([E_0; math_metrics])            # [1]
    out = avg + exp(-q)                                # scalar

Distribution (8 NeuronCores, no device collectives — launch skew across the
axon-tunneled cores makes any cross-core dependency cost 20-250us):
  Phase 1 (SPMD x8): column-shard W1: core j computes
      h1T_j = ReLU(x @ W1[:, 64j:64j+64] + b1[64j:64j+64]).T   -> [64, 11]
  Host: pure concatenation/layout shuffle of the 8 blocks (zero FLOPs).
  Phase 2 (1 core): layers 2..3 + loss tail on device -> scalar.
"""

import numpy as np

import concourse.bass as bass
import concourse.mybir as mybir
import concourse.tile as tile
from concourse import bacc
from concourse import bass_utils

F32 = mybir.dt.float32
N_CORES = 8
KDIM = 8192
EPS = 1e-8
ALU = mybir.AluOpType

# ---------------------------------------------------------------------------
# host-side layout helpers (pure reshapes/transposes, no FLOPs)
# ---------------------------------------------------------------------------


def _kmajor_image(a, p=128):
    """[K, M] (K = c*p + part) -> SBUF image [p, (K//p)*M]: img[part, c*M+m] = a[c*p+part, m]."""
    K, M = a.shape
    n = K // p
    return np.ascontiguousarray(a.reshape(n, p, M).transpose(1, 0, 2).reshape(p, n * M))


# ---------------------------------------------------------------------------
# phase 1: per-core first-layer column shard
# ---------------------------------------------------------------------------

N_W1_CHUNKS = 8
XC_COLS = 704 + 11 + 1  # xT image | ident [11,11] | b1 col


def build_phase1():
    nc = bacc.Bacc("TRN2", target_bir_lowering=False, debug=False,
                   num_devices=N_CORES)

    xc_img = nc.dram_tensor("xc_img", [128, XC_COLS], F32, kind="ExternalInput")
    w1_img = nc.dram_tensor("w1_img", [128, 64 * 64], F32, kind="ExternalInput")
    h1t_out = nc.dram_tensor("h1t_out", [64, 11], F32, kind="ExternalOutput")

    with tile.TileContext(nc) as tc:
        with (
            tc.tile_pool(name="sbuf", bufs=1) as sb,
            tc.tile_pool(name="psum", bufs=1, space="PSUM") as ps,
        ):
            xsb = sb.tile([128, XC_COLS], F32, tag="xsb")
            nc.sync.dma_start(xsb[:], xc_img[:])
            idsb = xsb[0:11, 704:715]
            b1sb = xsb[0:64, 715:716]

            w1sb = []
            cw = (64 * 64) // N_W1_CHUNKS  # 512 columns per chunk
            for c in range(N_W1_CHUNKS):
                t = sb.tile([128, cw], F32, tag=f"w1_{c}")
                eng = nc.sync if (c % 2 == 0) else nc.scalar
                eng.dma_start(t[:], w1_img[:, c * cw:(c + 1) * cw])
                w1sb.append(t)

            psum1 = ps.tile([11, 64], F32, tag="psum1")
            per_chunk = 64 // N_W1_CHUNKS
            for n in range(64):
                c, i = divmod(n, per_chunk)
                nc.tensor.matmul(
                    psum1[:, :],
                    xsb[:, 11 * n:11 * (n + 1)],
                    w1sb[c][:, 64 * i:64 * (i + 1)],
                    start=(n == 0),
                    stop=(n == 63),
                )

            pre = sb.tile([11, 64], F32, tag="pre")
            nc.vector.tensor_copy(pre[:], psum1[:])
            psT = ps.tile([64, 11], F32, tag="psT")
            nc.tensor.transpose(psT[:], pre[:], idsb)
            h1t = sb.tile([64, 11], F32, tag="h1t")
            # relu(x + b1) on DVE: (in + b1) max 0
            nc.vector.tensor_scalar(h1t[:], psT[:], b1sb, 0.0, ALU.add, ALU.max)
            nc.sync.dma_start(h1t_out[:], h1t[:])
    nc.compile()
    return nc


def phase1_inputs(x, W1, b1):
    """Per-core input maps for phase 1. x: [11, 8192]."""
    xc = np.zeros((128, XC_COLS), np.float32)
    xc[:, 0:704] = _kmajor_image(np.ascontiguousarray(x.T))
    xc[0:11, 704:715] = np.eye(11, dtype=np.float32)
    maps = []
    for j in range(N_CORES):
        w1j = np.ascontiguousarray(W1[:, 64 * j:64 * (j + 1)])  # [8192, 64]
        xcj = xc.copy()
        xcj[0:64, 715] = b1[64 * j:64 * (j + 1)]
        maps.append({
            "xc_img": xcj,
            "w1_img": _kmajor_image(w1j),                 # [128, 4096]
        })
    return maps


# ---------------------------------------------------------------------------
# phase 2: layers 2..3 + loss tail, single core
# ---------------------------------------------------------------------------

# const image layout (columns of imgB after the 512 W2 columns)
_CONST_SHAPES = [
    ("w3", 128, 256), ("b2", 128, 2), ("b3", 11, 128), ("ident", 11, 11),
    ("ones", 1, 11), ("wmean", 11, 1), ("wi1a", 128, 64), ("wi1b7", 7, 64),
    ("mm6e", 7, 1), ("wi2e", 65, 32), ("wi3e", 33, 1),
]
CONST_OFF = {}
_c = 0
for _n, _p, _f in _CONST_SHAPES:
    CONST_OFF[_n] = _c
    _c += _f
CONST_COLS = _c
IMGA_COLS = 44 + 512          # h1t image | W2 k-major cols 0..511
IMGB_COLS = 512 + CONST_COLS  # W2 k-major cols 512..1023 | consts


def build_phase2():
    nc = bacc.Bacc("TRN2", target_bir_lowering=False, debug=False, num_devices=1)

    imgA = nc.dram_tensor("imgA", [128, IMGA_COLS], F32, kind="ExternalInput")
    imgB = nc.dram_tensor("imgB", [128, IMGB_COLS], F32, kind="ExternalInput")
    out = nc.dram_tensor("out", [1, 1], F32, kind="ExternalOutput")

    AF = mybir.ActivationFunctionType

    with tile.TileContext(nc) as tc:
        with (
            tc.tile_pool(name="sbuf", bufs=1) as sb,
            tc.tile_pool(name="psum", bufs=2, space="PSUM") as ps,
        ):
            asb = sb.tile([128, IMGA_COLS], F32, tag="asb")
            nc.sync.dma_start(asb[:], imgA[:])
            bsb = sb.tile([128, IMGB_COLS], F32, tag="bsb")
            nc.scalar.dma_start(bsb[:], imgB[:])

            h1sb = asb[:, 0:44]

            def w2slice(t):  # [128, 256] k-major tile t of W2
                if t < 2:
                    return asb[:, 44 + 256 * t:44 + 256 * (t + 1)]
                return bsb[:, 256 * (t - 2):256 * (t - 1)]

            def cs(name, np_, nf):
                c0 = 512 + CONST_OFF[name]
                return bsb[0:np_, c0:c0 + nf]

            w3sb = cs("w3", 128, 256)
            b2sb = cs("b2", 128, 2)
            b3sb = cs("b3", 11, 128)
            idsb = cs("ident", 11, 11)
            onsb = cs("ones", 1, 11)
            wmsb = cs("wmean", 11, 1)
            wi1asb = cs("wi1a", 128, 64)
            wi1b7 = cs("wi1b7", 7, 64)
            mm6e = cs("mm6e", 7, 1)
            wi2e = cs("wi2e", 65, 32)
            wi3e = cs("wi3e", 33, 1)

            # hidden vectors with a trailing 1.0 partition (bias via K-extension)
            i1r = sb.tile([65, 1], F32, tag="i1r")
            nc.vector.memset(i1r[64:65, :], 1.0)
            i2r = sb.tile([33, 1], F32, tag="i2r")
            nc.vector.memset(i2r[32:33, :], 1.0)

            # ---- layer 2: h2 = ReLU(h1 @ W2 + b2), via h2T [2x(128,11)]
            psum2 = ps.tile([11, 256], F32, tag="pA")
            for t in range(4):
                nc.tensor.matmul(
                    psum2[:, :], h1sb[:, 11 * t:11 * (t + 1)], w2slice(t),
                    start=(t == 0), stop=(t == 3),
                )
            h2pre = sb.tile([11, 256], F32, tag="h2pre")
            nc.vector.tensor_copy(h2pre[:], psum2[:])
            h2t = sb.tile([128, 22], F32, tag="h2t")
            for t in range(2):
                pst = ps.tile([128, 11], F32, tag="pT")
                nc.tensor.transpose(pst[:], h2pre[:, 128 * t:128 * (t + 1)], idsb)
                nc.vector.tensor_scalar(h2t[:, 11 * t:11 * (t + 1)], pst[:],
                                        b2sb[:, t:t + 1], 0.0, ALU.add, ALU.max)

            # ---- layer 3: E = h2 @ W3 + b3  -> [11, 128]
            psum3 = ps.tile([11, 128], F32, tag="pA")
            for t in range(2):
                nc.tensor.matmul(
                    psum3[:, :], h2t[:, 11 * t:11 * (t + 1)],
                    w3sb[:, 128 * t:128 * (t + 1)],
                    start=(t == 0), stop=(t == 1),
                )
            Esb = sb.tile([11, 128], F32, tag="Esb")
            nc.vector.tensor_add(Esb[:], psum3[:], b3sb)

            # ---- broadcast row 0 (new_enc) to all 11 partitions
            bcN = ps.tile([11, 128], F32, tag="pA")
            nc.tensor.matmul(bcN[:, :], onsb, Esb[0:1, :], start=True, stop=True)

            # ---- per-row reductions (fused mult + row-sum)
            scr1 = sb.tile([11, 128], F32, tag="scr1")
            n2 = sb.tile([11, 1], F32, tag="n2")
            nc.vector.scalar_tensor_tensor(
                out=scr1[:], in0=Esb[:], scalar=1.0, in1=Esb[:],
                op0=ALU.mult, op1=ALU.mult, accum_out=n2[:])
            scr2 = sb.tile([11, 128], F32, tag="scr2")
            dvec = sb.tile([11, 1], F32, tag="dvec")
            nc.vector.scalar_tensor_tensor(
                out=scr2[:], in0=Esb[:], scalar=1.0, in1=bcN[:],
                op0=ALU.mult, op1=ALU.mult, accum_out=dvec[:])
            diff = sb.tile([11, 128], F32, tag="diff")
            nc.vector.tensor_sub(diff[:], Esb[:], bcN[:])
            scr3 = sb.tile([11, 128], F32, tag="scr3")
            g2 = sb.tile([11, 1], F32, tag="g2")
            nc.vector.scalar_tensor_tensor(
                out=scr3[:], in0=diff[:], scalar=1.0, in1=diff[:],
                op0=ALU.mult, op1=ALU.mult, accum_out=g2[:])

            # ---- norms, clamps, cos, geo
            nrm = sb.tile([11, 1], F32, tag="nrm")
            nc.scalar.activation(nrm[:], n2[:], AF.Sqrt)
            nmax = sb.tile([11, 1], F32, tag="nmax")
            nc.vector.tensor_scalar_max(nmax[:], nrm[:], EPS)
            geo = sb.tile([11, 1], F32, tag="geo")
            nc.scalar.activation(geo[:], g2[:], AF.Sqrt)

            bc0 = ps.tile([11, 1], F32, tag="pB")
            nc.tensor.matmul(bc0[:, :], onsb, nmax[0:1, 0:1], start=True, stop=True)
            denom = sb.tile([11, 1], F32, tag="denom")
            nc.vector.tensor_mul(denom[:], bc0[:], nmax[:])
            rden = sb.tile([11, 1], F32, tag="rden")
            nc.vector.reciprocal(rden[:], denom[:])
            cosv = sb.tile([11, 1], F32, tag="cosv")
            nc.vector.tensor_mul(cosv[:], dvec[:], rden[:])
            score = sb.tile([11, 1], F32, tag="score")
            nc.vector.tensor_sub(score[:], geo[:], cosv[:])

            # mean over rows 1..10 = wmean . score (wmean = [0, 0.1 x10])
            meanp = ps.tile([1, 1], F32, tag="pC")
            nc.tensor.matmul(meanp[:, :], score[:, 0:1], wmsb, start=True, stop=True)

            # ---- integrator MLP on [E_0; math_metrics], column form
            newT = ps.tile([128, 1], F32, tag="pB")
            nc.tensor.transpose(newT[:], Esb[0:1, :], idsb[0:1, 0:1])
            newTsb = sb.tile([128, 1], F32, tag="newTsb")
            nc.vector.tensor_copy(newTsb[:], newT[:])

            i1c = ps.tile([64, 1], F32, tag="pC")
            nc.tensor.matmul(i1c[:, :], wi1asb, newTsb[:, 0:1],
                             start=True, stop=False)
            nc.tensor.matmul(i1c[:, :], wi1b7, mm6e, start=False, stop=True)
            nc.vector.tensor_scalar_max(i1r[0:64, :], i1c[:, :], 0.0)

            i2c = ps.tile([32, 1], F32, tag="pC")
            nc.tensor.matmul(i2c[:, :], wi2e, i1r[:, 0:1], start=True, stop=True)
            nc.vector.tensor_scalar_max(i2r[0:32, :], i2c[:, :], 0.0)

            qp = ps.tile([1, 1], F32, tag="pC")
            nc.tensor.matmul(qp[:, :], wi3e, i2r[:, 0:1], start=True, stop=True)
            il = sb.tile([1, 1], F32, tag="il")
            nc.scalar.activation(il[:], qp[:], AF.Exp, scale=-1.0)

            total = sb.tile([1, 1], F32, tag="total")
            nc.vector.tensor_add(total[:], il[:], meanp[:])
            nc.sync.dma_start(out[:], total[:])
    nc.compile()
    return nc


def phase2_inputs(h1t_full, W2, b2, W3, b3, Wi1, bi1, Wi2, bi2, Wi3, bi3,
                  math_metrics):
    """h1t_full: [512, 11] = concat of the 8 per-core [64, 11] phase-1 outputs."""
    w2img = _kmajor_image(W2)  # [128, 1024]
    imgA = np.zeros((128, IMGA_COLS), np.float32)
    imgA[:, 0:44] = _kmajor_image(h1t_full)
    imgA[:, 44:556] = w2img[:, 0:512]

    wm = np.zeros((11, 1), np.float32)
    wm[1:, 0] = 0.1
    vals = {
        "w3": _kmajor_image(W3),
        "b2": b2.reshape(2, 128).T,
        "b3": np.tile(b3, (11, 1)),
        "ident": np.eye(11, dtype=np.float32),
        "ones": np.ones((1, 11), np.float32),
        "wmean": wm,
        "wi1a": Wi1[:128],
        "wi1b7": np.concatenate([Wi1[128:], bi1.reshape(1, 64)], axis=0),
        "mm6e": np.concatenate([math_metrics.reshape(6, 1),
                                np.ones((1, 1), np.float32)], axis=0),
        "wi2e": np.concatenate([Wi2, bi2.reshape(1, 32)], axis=0),
        "wi3e": np.concatenate([Wi3, bi3.reshape(1, 1)], axis=0),
    }
    imgB = np.zeros((128, IMGB_COLS), np.float32)
    imgB[:, 0:512] = w2img[:, 512:1024]
    for name, p, f in _CONST_SHAPES:
        v = np.asarray(vals[name], np.float32)
        assert v.shape == (p, f), (name, v.shape, (p, f))
        imgB[:p, 512 + CONST_OFF[name]:512 + CONST_OFF[name] + f] = v
    return {"imgA": imgA, "imgB": imgB}


# ---------------------------------------------------------------------------
# entry point
# ---------------------------------------------------------------------------

_NC1 = None
_NC2 = None


def _get_ncs():
    global _NC1, _NC2
    if _NC1 is None:
        _NC1 = build_phase1()
        _NC2 = build_phase2()
    return _NC1, _NC2


def kernel(new_knowledge, existing_knowledge, math_metrics,
           W1, b1, W2, b2, W3, b3, Wi1, bi1, Wi2, bi2, Wi3, bi3):
    args = [new_knowledge, existing_knowledge, math_metrics,
            W1, b1, W2, b2, W3, b3, Wi1, bi1, Wi2, bi2, Wi3, bi3]
    (new_knowledge, existing_knowledge, math_metrics,
     W1, b1, W2, b2, W3, b3, Wi1, bi1, Wi2, bi2, Wi3, bi3) = [
        np.asarray(a, np.float32) for a in args]

    nc1, nc2 = _get_ncs()

    x = np.concatenate([new_knowledge[None, :], existing_knowledge], axis=0)
    maps1 = phase1_inputs(x, W1, b1)
    res1 = bass_utils.run_bass_kernel_spmd(
        nc1, maps1, core_ids=list(range(N_CORES)))
    # pure gather: concat per-core transposed h1 blocks -> [512, 11]
    h1t_full = np.concatenate(
        [res1.results[j]["h1t_out"] for j in range(N_CORES)], axis=0)

    maps2 = [phase2_inputs(h1t_full, W2, b2, W3, b3,
                           Wi1, bi1, Wi2, bi2, Wi3, bi3, math_metrics)]
    res2 = bass_utils.run_bass_kernel_spmd(nc2, maps2, core_ids=[0])
    return res2.results[0]["out"].reshape(()).astype(np.float32)


# revision 7
# speedup vs baseline: 1.1144x; 1.0591x over previous
"""Trainium2 Bass kernel for nn_KnowledgeIntegrationLoss.

Computes (reference semantics):
    x = [new_knowledge; existing_knowledge]            # [11, 8192]
    E = MLP_encoder(x)                                 # [11, 128] (3 Linear, ReLU x2)
    geo_j = ||E_0 - E_j||, cos_j = <E_0,E_j>/(max(|E_0|,eps)*max(|E_j|,eps))
    avg = mean_{j=1..10}(geo_j - cos_j)
    q = MLP_integrator([E_0; math_metrics])            # [1]
    out = avg + exp(-q)                                # scalar

Distribution (8 NeuronCores, no device collectives — launch skew across the
axon-tunneled cores makes any cross-core dependency cost 20-250us):
  Phase 1 (SPMD x8): column-shard W1: core j computes
      h1T_j = ReLU(x @ W1[:, 64j:64j+64] + b1[64j:64j+64]).T   -> [64, 11]
  Host: pure concatenation/layout shuffle of the 8 blocks (zero FLOPs).
  Phase 2 (1 core): layers 2..3 + loss tail on device -> scalar.
"""

import numpy as np

import concourse.bass as bass
import concourse.mybir as mybir
import concourse.tile as tile
from concourse import bacc
from concourse import bass_utils

F32 = mybir.dt.float32
N_CORES = 8
KDIM = 8192
EPS = 1e-8
ALU = mybir.AluOpType

# ---------------------------------------------------------------------------
# host-side layout helpers (pure reshapes/transposes, no FLOPs)
# ---------------------------------------------------------------------------


def _kmajor_image(a, p=128):
    """[K, M] (K = c*p + part) -> SBUF image [p, (K//p)*M]: img[part, c*M+m] = a[c*p+part, m]."""
    K, M = a.shape
    n = K // p
    return np.ascontiguousarray(a.reshape(n, p, M).transpose(1, 0, 2).reshape(p, n * M))


# ---------------------------------------------------------------------------
# phase 1: per-core first-layer column shard
# ---------------------------------------------------------------------------

N_W1_CHUNKS = 8
XC_COLS = 704 + 11 + 1  # xT image | ident [11,11] | b1 col


def build_phase1():
    nc = bacc.Bacc("TRN2", target_bir_lowering=False, debug=False,
                   num_devices=N_CORES)

    xc_img = nc.dram_tensor("xc_img", [128, XC_COLS], F32, kind="ExternalInput")
    w1_img = nc.dram_tensor("w1_img", [128, 64 * 64], F32, kind="ExternalInput")
    h1t_out = nc.dram_tensor("h1t_out", [64, 11], F32, kind="ExternalOutput")

    with tile.TileContext(nc) as tc:
        with (
            tc.tile_pool(name="sbuf", bufs=1) as sb,
            tc.tile_pool(name="psum", bufs=1, space="PSUM") as ps,
        ):
            # x image alone on the Scalar HWDGE ring so its completion (which
            # gates every matmul) is not queued behind the 2.1MB of W1 traffic
            # on the Sync ring.
            xsb = sb.tile([128, XC_COLS], F32, tag="xsb")
            nc.scalar.dma_start(xsb[:], xc_img[:])
            idsb = xsb[0:11, 704:715]
            b1sb = xsb[0:64, 715:716]

            # W1 chunks on the Sync ring, in matmul consumption order; small
            # leading chunks so the first matmuls start early.
            w1sb = []
            chunk_ktiles = [8, 8, 8, 8, 16, 16]
            col0 = 0
            for c, kt in enumerate(chunk_ktiles):
                t = sb.tile([128, 64 * kt], F32, tag=f"w1_{c}")
                nc.sync.dma_start(t[:], w1_img[:, col0:col0 + 64 * kt])
                w1sb.append((t, col0 // 64))
                col0 += 64 * kt

            psum1 = ps.tile([11, 64], F32, tag="psum1")
            n = 0
            for t, n0 in w1sb:
                kt = t.shape[1] // 64
                for i in range(kt):
                    nc.tensor.matmul(
                        psum1[:, :],
                        xsb[:, 11 * n:11 * (n + 1)],
                        t[:, 64 * i:64 * (i + 1)],
                        start=(n == 0),
                        stop=(n == 63),
                    )
                    n += 1

            pre = sb.tile([11, 64], F32, tag="pre")
            nc.vector.tensor_copy(pre[:], psum1[:])
            psT = ps.tile([64, 11], F32, tag="psT")
            nc.tensor.transpose(psT[:], pre[:], idsb)
            h1t = sb.tile([64, 11], F32, tag="h1t")
            # relu(x + b1) on DVE: (in + b1) max 0
            nc.vector.tensor_scalar(h1t[:], psT[:], b1sb, 0.0, ALU.add, ALU.max)
            nc.sync.dma_start(h1t_out[:], h1t[:])
    nc.compile()
    return nc


def phase1_inputs(x, W1, b1):
    """Per-core input maps for phase 1. x: [11, 8192]."""
    xc = np.zeros((128, XC_COLS), np.float32)
    xc[:, 0:704] = _kmajor_image(np.ascontiguousarray(x.T))
    xc[0:11, 704:715] = np.eye(11, dtype=np.float32)
    maps = []
    for j in range(N_CORES):
        w1j = np.ascontiguousarray(W1[:, 64 * j:64 * (j + 1)])  # [8192, 64]
        xcj = xc.copy()
        xcj[0:64, 715] = b1[64 * j:64 * (j + 1)]
        maps.append({
            "xc_img": xcj,
            "w1_img": _kmajor_image(w1j),                 # [128, 4096]
        })
    return maps


# ---------------------------------------------------------------------------
# phase 2: layers 2..3 + loss tail, single core
# ---------------------------------------------------------------------------

# const image layout (columns of imgB after the 512 W2 columns)
_CONST_SHAPES = [
    ("w3", 128, 256), ("b2", 128, 2), ("b3", 11, 128), ("ident", 11, 11),
    ("ones", 1, 11), ("wmean", 11, 1), ("wi1a", 128, 64), ("wi1b7", 7, 64),
    ("mm6e", 7, 1), ("wi2e", 65, 32), ("wi3e", 33, 1),
]
CONST_OFF = {}
_c = 0
for _n, _p, _f in _CONST_SHAPES:
    CONST_OFF[_n] = _c
    _c += _f
CONST_COLS = _c
IMGA_COLS = 44 + 512          # h1t image | W2 k-major cols 0..511
IMGB_COLS = 512 + CONST_COLS  # W2 k-major cols 512..1023 | consts


def build_phase2():
    nc = bacc.Bacc("TRN2", target_bir_lowering=False, debug=False, num_devices=1)

    imgA = nc.dram_tensor("imgA", [128, IMGA_COLS], F32, kind="ExternalInput")
    imgB = nc.dram_tensor("imgB", [128, IMGB_COLS], F32, kind="ExternalInput")
    out = nc.dram_tensor("out", [1, 1], F32, kind="ExternalOutput")

    AF = mybir.ActivationFunctionType

    with tile.TileContext(nc) as tc:
        with (
            tc.tile_pool(name="sbuf", bufs=1) as sb,
            tc.tile_pool(name="psum", bufs=2, space="PSUM") as ps,
        ):
            asb = sb.tile([128, IMGA_COLS], F32, tag="asb")
            nc.sync.dma_start(asb[:], imgA[:])
            bsb = sb.tile([128, IMGB_COLS], F32, tag="bsb")
            nc.scalar.dma_start(bsb[:], imgB[:])

            h1sb = asb[:, 0:44]

            def w2slice(t):  # [128, 256] k-major tile t of W2
                if t < 2:
                    return asb[:, 44 + 256 * t:44 + 256 * (t + 1)]
                return bsb[:, 256 * (t - 2):256 * (t - 1)]

            def cs(name, np_, nf):
                c0 = 512 + CONST_OFF[name]
                return bsb[0:np_, c0:c0 + nf]

            w3sb = cs("w3", 128, 256)
            b2sb = cs("b2", 128, 2)
            b3sb = cs("b3", 11, 128)
            idsb = cs("ident", 11, 11)
            onsb = cs("ones", 1, 11)
            wmsb = cs("wmean", 11, 1)
            wi1asb = cs("wi1a", 128, 64)
            wi1b7 = cs("wi1b7", 7, 64)
            mm6e = cs("mm6e", 7, 1)
            wi2e = cs("wi2e", 65, 32)
            wi3e = cs("wi3e", 33, 1)

            # hidden vectors with a trailing 1.0 partition (bias via K-extension)
            i1r = sb.tile([65, 1], F32, tag="i1r")
            nc.vector.memset(i1r[64:65, :], 1.0)
            i2r = sb.tile([33, 1], F32, tag="i2r")
            nc.vector.memset(i2r[32:33, :], 1.0)

            # ---- layer 2: h2 = ReLU(h1 @ W2 + b2), via h2T [2x(128,11)]
            psum2 = ps.tile([11, 256], F32, tag="pA")
            for t in range(4):
                nc.tensor.matmul(
                    psum2[:, :], h1sb[:, 11 * t:11 * (t + 1)], w2slice(t),
                    start=(t == 0), stop=(t == 3),
                )
            h2pre = sb.tile([11, 256], F32, tag="h2pre")
            nc.vector.tensor_copy(h2pre[:], psum2[:])
            h2t = sb.tile([128, 22], F32, tag="h2t")
            for t in range(2):
                pst = ps.tile([128, 11], F32, tag="pT")
                nc.tensor.transpose(pst[:], h2pre[:, 128 * t:128 * (t + 1)], idsb)
                nc.vector.tensor_scalar(h2t[:, 11 * t:11 * (t + 1)], pst[:],
                                        b2sb[:, t:t + 1], 0.0, ALU.add, ALU.max)

            # ---- layer 3: E = h2 @ W3 + b3  -> [11, 128]
            psum3 = ps.tile([11, 128], F32, tag="pA")
            for t in range(2):
                nc.tensor.matmul(
                    psum3[:, :], h2t[:, 11 * t:11 * (t + 1)],
                    w3sb[:, 128 * t:128 * (t + 1)],
                    start=(t == 0), stop=(t == 1),
                )
            Esb = sb.tile([11, 128], F32, tag="Esb")
            nc.vector.tensor_add(Esb[:], psum3[:], b3sb)

            # ---- broadcast row 0 (new_enc) to all 11 partitions
            bcN = ps.tile([11, 128], F32, tag="pA")
            nc.tensor.matmul(bcN[:, :], onsb, Esb[0:1, :], start=True, stop=True)

            # ---- per-row reductions (fused mult + row-sum)
            scr1 = sb.tile([11, 128], F32, tag="scr1")
            n2 = sb.tile([11, 1], F32, tag="n2")
            nc.vector.scalar_tensor_tensor(
                out=scr1[:], in0=Esb[:], scalar=1.0, in1=Esb[:],
                op0=ALU.mult, op1=ALU.mult, accum_out=n2[:])
            scr2 = sb.tile([11, 128], F32, tag="scr2")
            dvec = sb.tile([11, 1], F32, tag="dvec")
            nc.vector.scalar_tensor_tensor(
                out=scr2[:], in0=Esb[:], scalar=1.0, in1=bcN[:],
                op0=ALU.mult, op1=ALU.mult, accum_out=dvec[:])
            diff = sb.tile([11, 128], F32, tag="diff")
            nc.vector.tensor_sub(diff[:], Esb[:], bcN[:])
            scr3 = sb.tile([11, 128], F32, tag="scr3")
            g2 = sb.tile([11, 1], F32, tag="g2")
            nc.vector.scalar_tensor_tensor(
                out=scr3[:], in0=diff[:], scalar=1.0, in1=diff[:],
                op0=ALU.mult, op1=ALU.mult, accum_out=g2[:])

            # ---- norms, clamps, cos, geo
            nrm = sb.tile([11, 1], F32, tag="nrm")
            nc.scalar.activation(nrm[:], n2[:], AF.Sqrt)
            nmax = sb.tile([11, 1], F32, tag="nmax")
            nc.vector.tensor_scalar_max(nmax[:], nrm[:], EPS)
            geo = sb.tile([11, 1], F32, tag="geo")
            nc.scalar.activation(geo[:], g2[:], AF.Sqrt)

            bc0 = ps.tile([11, 1], F32, tag="pB")
            nc.tensor.matmul(bc0[:, :], onsb, nmax[0:1, 0:1], start=True, stop=True)
            denom = sb.tile([11, 1], F32, tag="denom")
            nc.vector.tensor_mul(denom[:], bc0[:], nmax[:])
            rden = sb.tile([11, 1], F32, tag="rden")
            nc.vector.reciprocal(rden[:], denom[:])
            cosv = sb.tile([11, 1], F32, tag="cosv")
            nc.vector.tensor_mul(cosv[:], dvec[:], rden[:])
            score = sb.tile([11, 1], F32, tag="score")
            nc.vector.tensor_sub(score[:], geo[:], cosv[:])

            # mean over rows 1..10 = wmean . score (wmean = [0, 0.1 x10])
            meanp = ps.tile([1, 1], F32, tag="pC")
            nc.tensor.matmul(meanp[:, :], score[:, 0:1], wmsb, start=True, stop=True)

            # ---- integrator MLP on [E_0; math_metrics], column form
            newT = ps.tile([128, 1], F32, tag="pB")
            nc.tensor.transpose(newT[:], Esb[0:1, :], idsb[0:1, 0:1])
            newTsb = sb.tile([128, 1], F32, tag="newTsb")
            nc.vector.tensor_copy(newTsb[:], newT[:])

            i1c = ps.tile([64, 1], F32, tag="pC")
            nc.tensor.matmul(i1c[:, :], wi1asb, newTsb[:, 0:1],
                             start=True, stop=False)
            nc.tensor.matmul(i1c[:, :], wi1b7, mm6e, start=False, stop=True)
            nc.vector.tensor_scalar_max(i1r[0:64, :], i1c[:, :], 0.0)

            i2c = ps.tile([32, 1], F32, tag="pC")
            nc.tensor.matmul(i2c[:, :], wi2e, i1r[:, 0:1], start=True, stop=True)
            nc.vector.tensor_scalar_max(i2r[0:32, :], i2c[:, :], 0.0)

            qp = ps.tile([1, 1], F32, tag="pC")
            nc.tensor.matmul(qp[:, :], wi3e, i2r[:, 0:1], start=True, stop=True)
            il = sb.tile([1, 1], F32, tag="il")
            nc.scalar.activation(il[:], qp[:], AF.Exp, scale=-1.0)

            total = sb.tile([1, 1], F32, tag="total")
            nc.vector.tensor_add(total[:], il[:], meanp[:])
            nc.sync.dma_start(out[:], total[:])
    nc.compile()
    return nc


def phase2_inputs(h1t_full, W2, b2, W3, b3, Wi1, bi1, Wi2, bi2, Wi3, bi3,
                  math_metrics):
    """h1t_full: [512, 11] = concat of the 8 per-core [64, 11] phase-1 outputs."""
    w2img = _kmajor_image(W2)  # [128, 1024]
    imgA = np.zeros((128, IMGA_COLS), np.float32)
    imgA[:, 0:44] = _kmajor_image(h1t_full)
    imgA[:, 44:556] = w2img[:, 0:512]

    wm = np.zeros((11, 1), np.float32)
    wm[1:, 0] = 0.1
    vals = {
        "w3": _kmajor_image(W3),
        "b2": b2.reshape(2, 128).T,
        "b3": np.tile(b3, (11, 1)),
        "ident": np.eye(11, dtype=np.float32),
        "ones": np.ones((1, 11), np.float32),
        "wmean": wm,
        "wi1a": Wi1[:128],
        "wi1b7": np.concatenate([Wi1[128:], bi1.reshape(1, 64)], axis=0),
        "mm6e": np.concatenate([math_metrics.reshape(6, 1),
                                np.ones((1, 1), np.float32)], axis=0),
        "wi2e": np.concatenate([Wi2, bi2.reshape(1, 32)], axis=0),
        "wi3e": np.concatenate([Wi3, bi3.reshape(1, 1)], axis=0),
    }
    imgB = np.zeros((128, IMGB_COLS), np.float32)
    imgB[:, 0:512] = w2img[:, 512:1024]
    for name, p, f in _CONST_SHAPES:
        v = np.asarray(vals[name], np.float32)
        assert v.shape == (p, f), (name, v.shape, (p, f))
        imgB[:p, 512 + CONST_OFF[name]:512 + CONST_OFF[name] + f] = v
    return {"imgA": imgA, "imgB": imgB}


# ---------------------------------------------------------------------------
# entry point
# ---------------------------------------------------------------------------

_NC1 = None
_NC2 = None


def _get_ncs():
    global _NC1, _NC2
    if _NC1 is None:
        _NC1 = build_phase1()
        _NC2 = build_phase2()
    return _NC1, _NC2


def kernel(new_knowledge, existing_knowledge, math_metrics,
           W1, b1, W2, b2, W3, b3, Wi1, bi1, Wi2, bi2, Wi3, bi3):
    args = [new_knowledge, existing_knowledge, math_metrics,
            W1, b1, W2, b2, W3, b3, Wi1, bi1, Wi2, bi2, Wi3, bi3]
    (new_knowledge, existing_knowledge, math_metrics,
     W1, b1, W2, b2, W3, b3, Wi1, bi1, Wi2, bi2, Wi3, bi3) = [
        np.asarray(a, np.float32) for a in args]

    nc1, nc2 = _get_ncs()

    x = np.concatenate([new_knowledge[None, :], existing_knowledge], axis=0)
    maps1 = phase1_inputs(x, W1, b1)
    res1 = bass_utils.run_bass_kernel_spmd(
        nc1, maps1, core_ids=list(range(N_CORES)))
    # pure gather: concat per-core transposed h1 blocks -> [512, 11]
    h1t_full = np.concatenate(
        [res1.results[j]["h1t_out"] for j in range(N_CORES)], axis=0)

    maps2 = [phase2_inputs(h1t_full, W2, b2, W3, b3,
                           Wi1, bi1, Wi2, bi2, Wi3, bi3, math_metrics)]
    res2 = bass_utils.run_bass_kernel_spmd(nc2, maps2, core_ids=[0])
    return res2.results[0]["out"].reshape(()).astype(np.float32)


# revision 9
# speedup vs baseline: 1.1273x; 1.0116x over previous
"""Trainium2 Bass kernel for nn_KnowledgeIntegrationLoss.

Computes (reference semantics):
    x = [new_knowledge; existing_knowledge]            # [11, 8192]
    E = MLP_encoder(x)                                 # [11, 128] (3 Linear, ReLU x2)
    geo_j = ||E_0 - E_j||, cos_j = <E_0,E_j>/(max(|E_0|,eps)*max(|E_j|,eps))
    avg = mean_{j=1..10}(geo_j - cos_j)
    q = MLP_integrator([E_0; math_metrics])            # [1]
    out = avg + exp(-q)                                # scalar

Distribution (8 NeuronCores, no device collectives — launch skew across the
axon-tunneled cores makes any cross-core dependency cost 20-250us):
  Phase 1 (SPMD x8): column-shard W1: core j computes
      h1T_j = ReLU(x @ W1[:, 64j:64j+64] + b1[64j:64j+64]).T   -> [64, 11]
  Host: pure concatenation/layout shuffle of the 8 blocks (zero FLOPs).
  Phase 2 (1 core): layers 2..3 + loss tail on device -> scalar.
"""

import numpy as np

import concourse.bass as bass
import concourse.mybir as mybir
import concourse.tile as tile
from concourse import bacc
from concourse import bass_utils

F32 = mybir.dt.float32
N_CORES = 8
KDIM = 8192
EPS = 1e-8
ALU = mybir.AluOpType

# ---------------------------------------------------------------------------
# host-side layout helpers (pure reshapes/transposes, no FLOPs)
# ---------------------------------------------------------------------------


def _kmajor_image(a, p=128):
    """[K, M] (K = c*p + part) -> SBUF image [p, (K//p)*M]: img[part, c*M+m] = a[c*p+part, m]."""
    K, M = a.shape
    n = K // p
    return np.ascontiguousarray(a.reshape(n, p, M).transpose(1, 0, 2).reshape(p, n * M))


# ---------------------------------------------------------------------------
# phase 1: per-core first-layer column shard
# ---------------------------------------------------------------------------

N_W1_CHUNKS = 8
XC_COLS = 704 + 11 + 1  # xT image | ident [11,11] | b1 col


def build_phase1():
    nc = bacc.Bacc("TRN2", target_bir_lowering=False, debug=False,
                   num_devices=N_CORES)

    xc_img = nc.dram_tensor("xc_img", [128, XC_COLS], F32, kind="ExternalInput")
    w1_img = nc.dram_tensor("w1_img", [128, 64 * 64], F32, kind="ExternalInput")
    h1t_out = nc.dram_tensor("h1t_out", [64, 11], F32, kind="ExternalOutput")

    with tile.TileContext(nc) as tc:
        with (
            tc.tile_pool(name="sbuf", bufs=1) as sb,
            tc.tile_pool(name="psum", bufs=1, space="PSUM") as ps,
        ):
            # x image first on the Scalar HWDGE ring so its completion (which
            # gates every matmul) is not queued behind the 2.1MB of W1 traffic;
            # W1 chunks split across both rings in matmul consumption order.
            xsb = sb.tile([128, XC_COLS], F32, tag="xsb")
            nc.scalar.dma_start(xsb[:], xc_img[:])
            idsb = xsb[0:11, 704:715]
            b1sb = xsb[0:64, 715:716]

            chunk_plan = [  # (k-tiles, engine) in consumption order
                (8, nc.sync), (8, nc.sync), (8, nc.sync), (8, nc.scalar),
                (16, nc.sync), (16, nc.scalar),
            ]
            w1sb = []
            col0 = 0
            for c, (kt, eng) in enumerate(chunk_plan):
                t = sb.tile([128, 64 * kt], F32, tag=f"w1_{c}")
                eng.dma_start(t[:], w1_img[:, col0:col0 + 64 * kt])
                w1sb.append(t)
                col0 += 64 * kt

            # PE warm-up: junk matmuls on a memset tile keep the HAM activity
            # window busy so the real matmuls run at 2.4GHz instead of 1.2.
            junk = sb.tile([128, 64], F32, tag="junk")
            nc.gpsimd.memset(junk[:], 0.0)
            jps = ps.tile([11, 64], F32, tag="jps")
            for _ in range(48):
                nc.tensor.matmul(jps[:, :], junk[:, 0:11], junk[:, :],
                                 start=True, stop=True)

            psum1 = ps.tile([11, 64], F32, tag="psum1")
            n = 0
            for t in w1sb:
                kt = t.shape[1] // 64
                for i in range(kt):
                    nc.tensor.matmul(
                        psum1[:, :],
                        xsb[:, 11 * n:11 * (n + 1)],
                        t[:, 64 * i:64 * (i + 1)],
                        start=(n == 0),
                        stop=(n == 63),
                    )
                    n += 1

            pre = sb.tile([11, 64], F32, tag="pre")
            nc.vector.tensor_copy(pre[:], psum1[:])
            psT = ps.tile([64, 11], F32, tag="psT")
            nc.tensor.transpose(psT[:], pre[:], idsb)
            h1t = sb.tile([64, 11], F32, tag="h1t")
            # relu(x + b1) on DVE: (in + b1) max 0
            nc.vector.tensor_scalar(h1t[:], psT[:], b1sb, 0.0, ALU.add, ALU.max)
            nc.sync.dma_start(h1t_out[:], h1t[:])
    nc.compile()
    return nc


def phase1_inputs(x, W1, b1):
    """Per-core input maps for phase 1. x: [11, 8192]."""
    xc = np.zeros((128, XC_COLS), np.float32)
    xc[:, 0:704] = _kmajor_image(np.ascontiguousarray(x.T))
    xc[0:11, 704:715] = np.eye(11, dtype=np.float32)
    maps = []
    for j in range(N_CORES):
        w1j = np.ascontiguousarray(W1[:, 64 * j:64 * (j + 1)])  # [8192, 64]
        xcj = xc.copy()
        xcj[0:64, 715] = b1[64 * j:64 * (j + 1)]
        maps.append({
            "xc_img": xcj,
            "w1_img": _kmajor_image(w1j),                 # [128, 4096]
        })
    return maps


# ---------------------------------------------------------------------------
# phase 2: layers 2..3 + loss tail, single core
# ---------------------------------------------------------------------------

# const image layout (columns of imgB after the 512 W2 columns)
_CONST_SHAPES = [
    ("w3", 128, 256), ("b2", 128, 2), ("b3", 11, 128), ("b3c", 128, 1),
    ("ones", 1, 11), ("wmean", 11, 1), ("wi1a", 128, 64), ("wi1b7", 7, 64),
    ("mm6e", 7, 1), ("wi2e", 65, 32), ("wi3e", 33, 1),
]
CONST_OFF = {}
_c = 0
for _n, _p, _f in _CONST_SHAPES:
    CONST_OFF[_n] = _c
    _c += _f
CONST_COLS = _c
IMGA_COLS = 44 + 512          # h1t image | W2 k-major cols 0..511
IMGB_COLS = 512 + CONST_COLS  # W2 k-major cols 512..1023 | consts


def build_phase2():
    nc = bacc.Bacc("TRN2", target_bir_lowering=False, debug=False, num_devices=1)

    imgA = nc.dram_tensor("imgA", [128, IMGA_COLS], F32, kind="ExternalInput")
    imgB = nc.dram_tensor("imgB", [128, IMGB_COLS], F32, kind="ExternalInput")
    out = nc.dram_tensor("out", [1, 1], F32, kind="ExternalOutput")

    AF = mybir.ActivationFunctionType

    with tile.TileContext(nc) as tc:
        with (
            tc.tile_pool(name="sbuf", bufs=1) as sb,
            tc.tile_pool(name="psum", bufs=1, space="PSUM") as ps,
        ):
            asb = sb.tile([128, IMGA_COLS], F32, tag="asb")
            nc.sync.dma_start(asb[:], imgA[:])
            bsb = sb.tile([128, IMGB_COLS], F32, tag="bsb")
            nc.scalar.dma_start(bsb[:], imgB[:])

            h1sb = asb[:, 0:44]

            def w2l(t, h):  # lhsT [128, 128]: W2[128t+p, 128h+m]
                if t < 2:
                    return asb[:, 44 + 256 * t + 128 * h:44 + 256 * t + 128 * (h + 1)]
                return bsb[:, 256 * (t - 2) + 128 * h:256 * (t - 2) + 128 * (h + 1)]

            def cs(name, np_, nf):
                c0 = 512 + CONST_OFF[name]
                return bsb[0:np_, c0:c0 + nf]

            w3sb = cs("w3", 128, 256)
            b2sb = cs("b2", 128, 2)
            b3sb = cs("b3", 11, 128)
            b3c = cs("b3c", 128, 1)
            onsb = cs("ones", 1, 11)
            wmsb = cs("wmean", 11, 1)
            wi1asb = cs("wi1a", 128, 64)
            wi1b7 = cs("wi1b7", 7, 64)
            mm6e = cs("mm6e", 7, 1)
            wi2e = cs("wi2e", 65, 32)
            wi3e = cs("wi3e", 33, 1)

            # hidden vectors with a trailing 1.0 partition (bias via K-extension)
            i1r = sb.tile([65, 1], F32, tag="i1r")
            nc.vector.memset(i1r[64:65, :], 1.0)
            i2r = sb.tile([33, 1], F32, tag="i2r")
            nc.vector.memset(i2r[32:33, :], 1.0)

            # ---- layer 2 direct to h2T: psum2T_h [128, 11] = (h1 @ W2).T half h
            h2t = sb.tile([128, 22], F32, tag="h2t")
            for h in range(2):
                p2t = ps.tile([128, 11], F32, tag="pA", bufs=3)
                for t in range(4):
                    nc.tensor.matmul(
                        p2t[:, :], w2l(t, h), h1sb[:, 11 * t:11 * (t + 1)],
                        start=(t == 0), stop=(t == 3),
                    )
                # relu(x + b2) on DVE
                nc.vector.tensor_scalar(h2t[:, 11 * h:11 * (h + 1)], p2t[:],
                                        b2sb[:, h:h + 1], 0.0, ALU.add, ALU.max)

            # ---- layer 3 row form: E = h2 @ W3 + b3 -> [11, 128]
            psum3 = ps.tile([11, 128], F32, tag="pA", bufs=3)
            for h in range(2):
                nc.tensor.matmul(
                    psum3[:, :], h2t[:, 11 * h:11 * (h + 1)],
                    w3sb[:, 128 * h:128 * (h + 1)],
                    start=(h == 0), stop=(h == 1),
                )
            Esb = sb.tile([11, 128], F32, tag="Esb")
            nc.vector.tensor_add(Esb[:], psum3[:], b3sb)

            # ---- layer 3 col form (only column 0 needed): E_0^T [128, 1]
            psET = ps.tile([128, 11], F32, tag="pA", bufs=3)
            for h in range(2):
                nc.tensor.matmul(
                    psET[:, :], w3sb[:, 128 * h:128 * (h + 1)],
                    h2t[:, 11 * h:11 * (h + 1)],
                    start=(h == 0), stop=(h == 1),
                )
            newTsb = sb.tile([128, 1], F32, tag="newTsb")
            nc.vector.tensor_scalar(newTsb[:], psET[:, 0:1], b3c, 0.0,
                                    ALU.add, ALU.bypass)

            # ---- integrator MLP on [E_0; math_metrics], column form
            i1c = ps.tile([64, 1], F32, tag="pC", bufs=2)
            nc.tensor.matmul(i1c[:, :], wi1asb, newTsb[:, 0:1],
                             start=True, stop=False)
            nc.tensor.matmul(i1c[:, :], wi1b7, mm6e, start=False, stop=True)
            nc.vector.tensor_scalar_max(i1r[0:64, :], i1c[:, :], 0.0)

            i2c = ps.tile([32, 1], F32, tag="pC", bufs=2)
            nc.tensor.matmul(i2c[:, :], wi2e, i1r[:, 0:1], start=True, stop=True)
            nc.vector.tensor_scalar_max(i2r[0:32, :], i2c[:, :], 0.0)

            qp = ps.tile([1, 1], F32, tag="pC", bufs=2)
            nc.tensor.matmul(qp[:, :], wi3e, i2r[:, 0:1], start=True, stop=True)
            il = sb.tile([1, 1], F32, tag="il")
            nc.scalar.activation(il[:], qp[:], AF.Exp, scale=-1.0)

            # ---- broadcast row 0 (new_enc) to all 11 partitions
            bcN = ps.tile([11, 128], F32, tag="pA", bufs=3)
            nc.tensor.matmul(bcN[:, :], onsb, Esb[0:1, :], start=True, stop=True)

            # ---- per-row reductions (fused mult + row-sum)
            scr1 = sb.tile([11, 128], F32, tag="scr1")
            n2 = sb.tile([11, 1], F32, tag="n2")
            nc.vector.scalar_tensor_tensor(
                out=scr1[:], in0=Esb[:], scalar=1.0, in1=Esb[:],
                op0=ALU.mult, op1=ALU.mult, accum_out=n2[:])
            scr2 = sb.tile([11, 128], F32, tag="scr2")
            dvec = sb.tile([11, 1], F32, tag="dvec")
            nc.vector.scalar_tensor_tensor(
                out=scr2[:], in0=Esb[:], scalar=1.0, in1=bcN[:],
                op0=ALU.mult, op1=ALU.mult, accum_out=dvec[:])
            diff = sb.tile([11, 128], F32, tag="diff")
            nc.vector.tensor_sub(diff[:], Esb[:], bcN[:])
            scr3 = sb.tile([11, 128], F32, tag="scr3")
            g2 = sb.tile([11, 1], F32, tag="g2")
            nc.vector.scalar_tensor_tensor(
                out=scr3[:], in0=diff[:], scalar=1.0, in1=diff[:],
                op0=ALU.mult, op1=ALU.mult, accum_out=g2[:])

            # ---- norms, clamps, cos, geo
            nrm = sb.tile([11, 1], F32, tag="nrm")
            nc.scalar.activation(nrm[:], n2[:], AF.Sqrt)
            nmax = sb.tile([11, 1], F32, tag="nmax")
            nc.vector.tensor_scalar_max(nmax[:], nrm[:], EPS)
            geo = sb.tile([11, 1], F32, tag="geo")
            nc.scalar.activation(geo[:], g2[:], AF.Sqrt)

            bc0 = ps.tile([11, 1], F32, tag="pB")
            nc.tensor.matmul(bc0[:, :], onsb, nmax[0:1, 0:1], start=True, stop=True)
            denom = sb.tile([11, 1], F32, tag="denom")
            nc.vector.tensor_mul(denom[:], bc0[:], nmax[:])
            rden = sb.tile([11, 1], F32, tag="rden")
            nc.vector.reciprocal(rden[:], denom[:])
            # negscore = cos - geo (fused); mean uses negated weights
            negscore = sb.tile([11, 1], F32, tag="negscore")
            nc.vector.scalar_tensor_tensor(
                out=negscore[:], in0=dvec[:], scalar=rden[0:11, 0:1],
                in1=geo[:], op0=ALU.mult, op1=ALU.subtract)

            # mean over rows 1..10 = wmean . negscore (wmean = [0, -0.1 x10])
            meanp = ps.tile([1, 1], F32, tag="pC", bufs=2)
            nc.tensor.matmul(meanp[:, :], negscore[:, 0:1], wmsb,
                             start=True, stop=True)

            total = sb.tile([1, 1], F32, tag="total")
            nc.vector.tensor_add(total[:], il[:], meanp[:])
            nc.sync.dma_start(out[:], total[:])
    nc.compile()
    return nc


def phase2_inputs(h1t_full, W2, b2, W3, b3, Wi1, bi1, Wi2, bi2, Wi3, bi3,
                  math_metrics):
    """h1t_full: [512, 11] = concat of the 8 per-core [64, 11] phase-1 outputs."""
    w2img = _kmajor_image(W2)  # [128, 1024]
    imgA = np.zeros((128, IMGA_COLS), np.float32)
    imgA[:, 0:44] = _kmajor_image(h1t_full)
    imgA[:, 44:556] = w2img[:, 0:512]

    wm = np.zeros((11, 1), np.float32)
    wm[1:, 0] = -0.1
    vals = {
        "w3": _kmajor_image(W3),
        "b2": b2.reshape(2, 128).T,
        "b3": np.tile(b3, (11, 1)),
        "b3c": b3.reshape(128, 1),
        "ones": np.ones((1, 11), np.float32),
        "wmean": wm,
        "wi1a": Wi1[:128],
        "wi1b7": np.concatenate([Wi1[128:], bi1.reshape(1, 64)], axis=0),
        "mm6e": np.concatenate([math_metrics.reshape(6, 1),
                                np.ones((1, 1), np.float32)], axis=0),
        "wi2e": np.concatenate([Wi2, bi2.reshape(1, 32)], axis=0),
        "wi3e": np.concatenate([Wi3, bi3.reshape(1, 1)], axis=0),
    }
    imgB = np.zeros((128, IMGB_COLS), np.float32)
    imgB[:, 0:512] = w2img[:, 512:1024]
    for name, p, f in _CONST_SHAPES:
        v = np.asarray(vals[name], np.float32)
        assert v.shape == (p, f), (name, v.shape, (p, f))
        imgB[:p, 512 + CONST_OFF[name]:512 + CONST_OFF[name] + f] = v
    return {"imgA": imgA, "imgB": imgB}


# ---------------------------------------------------------------------------
# entry point
# ---------------------------------------------------------------------------

_NC1 = None
_NC2 = None


def _get_ncs():
    global _NC1, _NC2
    if _NC1 is None:
        _NC1 = build_phase1()
        _NC2 = build_phase2()
    return _NC1, _NC2


def kernel(new_knowledge, existing_knowledge, math_metrics,
           W1, b1, W2, b2, W3, b3, Wi1, bi1, Wi2, bi2, Wi3, bi3):
    args = [new_knowledge, existing_knowledge, math_metrics,
            W1, b1, W2, b2, W3, b3, Wi1, bi1, Wi2, bi2, Wi3, bi3]
    (new_knowledge, existing_knowledge, math_metrics,
     W1, b1, W2, b2, W3, b3, Wi1, bi1, Wi2, bi2, Wi3, bi3) = [
        np.asarray(a, np.float32) for a in args]

    nc1, nc2 = _get_ncs()

    x = np.concatenate([new_knowledge[None, :], existing_knowledge], axis=0)
    maps1 = phase1_inputs(x, W1, b1)
    res1 = bass_utils.run_bass_kernel_spmd(
        nc1, maps1, core_ids=list(range(N_CORES)))
    # pure gather: concat per-core transposed h1 blocks -> [512, 11]
    h1t_full = np.concatenate(
        [res1.results[j]["h1t_out"] for j in range(N_CORES)], axis=0)

    maps2 = [phase2_inputs(h1t_full, W2, b2, W3, b3,
                           Wi1, bi1, Wi2, bi2, Wi3, bi3, math_metrics)]
    res2 = bass_utils.run_bass_kernel_spmd(nc2, maps2, core_ids=[0])
    return res2.results[0]["out"].reshape(()).astype(np.float32)
